# revision 3
# baseline (speedup 1.0000x reference)
"""ArcFace loss on 8 TRN2 NeuronCores — class-dimension (C) sharded.

Math (reference has M1=1, M2=0.5, M3=0, scale=64, label_smoothing=0):
  per row i with one-hot y_true:  v_i = x[i, label_i] = sum_j y[i,j]*x[i,j]
  t_i = cos(acos(v_i) + 0.5),  t_i -> -2 - t_i when v_i <= cos(pi - 0.5)
  loss_i = logsumexp_j(64 * modified_x[i,j]) - 64*t_i
  loss = mean_i loss_i          (0 when a row of y_true is all zero)

All logits lie in (-0.99, 0.99), so 64*x - 64 <= 0 and a FIXED shift of 64
replaces the row-max in logsumexp (no max pass, no second streaming pass):
  logsumexp_i = 64 + log(S_i),
  S_i = sum_j exp(64*x[i,j] - 64) + exp(64*t_i - 64) - exp(64*v_i - 64)

Each core streams its [512, 12500] shard of x and y once and emits per-row
partials:
  hvh_i = sum_j (x[i,j] + 16) * y[i,j]   (= v_i + 16 if the label is local,
                                          exactly 0 otherwise — encodes both
                                          the hit flag and the hit value)
  se_i  = sum_j exp(64*x[i,j] - 64)
plus column 0 of the local shard (needed to mimic argmax(all-zeros)=0 when a
y_true row is entirely zero — the reference then returns a 0 contribution,
so col0 is only used to keep the formulas well-defined).

The host "unshard" step sums the [512]-sized partials over the 8 cores and
applies the closed-form tail (acos/cos/log on 512 scalars).
"""

import numpy as np

B = 512
C = 100000
NCORES = 8
CS = C // NCORES  # 12500 classes per core
P = 128
RG = B // P  # 4 row groups of 128 partitions
FCH = 3125  # free-dim chunk
NCH = CS // FCH  # 4 chunks per row group

KOFF = 16.0  # hit-encoding offset: hvh = v + 16 iff label is in-shard
SCALE = 64.0
M2 = 0.5
THRESHOLD = float(np.cos(np.pi - M2))

_CACHE = {}


def _build_nc():
    import concourse.tile as tile
    from concourse import bacc, mybir

    nc = bacc.Bacc(
        "TRN2",
        target_bir_lowering=False,
        debug=False,
        enable_asserts=False,
        num_devices=NCORES,
    )
    f32 = mybir.dt.float32
    x_d = nc.dram_tensor("x", [B, CS], f32, kind="ExternalInput").ap()
    y_d = nc.dram_tensor("y", [B, CS], f32, kind="ExternalInput").ap()
    # out columns: [0:RG] hvh per row group, [RG:2RG] se, [2RG:3RG] shard col0
    out_d = nc.dram_tensor("out", [P, 3 * RG], f32, kind="ExternalOutput").ap()

    with tile.TileContext(nc) as tc:
        with (
            tc.tile_pool(name="xin", bufs=3) as xpool,
            tc.tile_pool(name="yin", bufs=3) as ypool,
            tc.tile_pool(name="escratch", bufs=2) as epool,
            tc.tile_pool(name="stats", bufs=1) as stats,
        ):
            hvh_parts = stats.tile([P, RG * NCH], f32)
            se_parts = stats.tile([P, RG * NCH], f32)
            outsb = stats.tile([P, 3 * RG], f32)
            dummy = stats.tile([P, 1], f32)
            neg_scale = stats.tile([P, 1], f32)
            nc.vector.memset(neg_scale[:], -SCALE)

            for r in range(RG):
                for c in range(NCH):
                    i = r * NCH + c
                    xt = xpool.tile([P, FCH], f32)
                    nc.sync.dma_start(
                        xt[:], x_d[r * P : (r + 1) * P, c * FCH : (c + 1) * FCH]
                    )
                    yt = ypool.tile([P, FCH], f32)
                    nc.sync.dma_start(
                        yt[:], y_d[r * P : (r + 1) * P, c * FCH : (c + 1) * FCH]
                    )
                    # DVE: hvh partial = sum((x + 16) * y) along the chunk
                    nc.vector.scalar_tensor_tensor(
                        out=dummy.broadcast_to([P, FCH]),
                        in0=xt[:],
                        scalar=KOFF,
                        in1=yt[:],
                        op0=mybir.AluOpType.add,
                        op1=mybir.AluOpType.mult,
                        accum_out=hvh_parts[:, i : i + 1],
                    )
                    # ACT: se partial = sum(exp(64*x - 64)) along the chunk
                    et = epool.tile([P, FCH], f32)
                    nc.scalar.activation(
                        out=et[:],
                        in_=xt[:],
                        func=mybir.ActivationFunctionType.Exp,
                        bias=neg_scale[:],
                        scale=SCALE,
                        accum_out=se_parts[:, i : i + 1],
                    )
                    if c == 0:
                        nc.vector.tensor_copy(
                            outsb[:, 2 * RG + r : 2 * RG + r + 1], xt[:, 0:1]
                        )

            for r in range(RG):
                nc.vector.tensor_reduce(
                    out=outsb[:, r : r + 1],
                    in_=hvh_parts[:, r * NCH : (r + 1) * NCH],
                    axis=mybir.AxisListType.X,
                    op=mybir.AluOpType.add,
                )
                nc.vector.tensor_reduce(
                    out=outsb[:, RG + r : RG + r + 1],
                    in_=se_parts[:, r * NCH : (r + 1) * NCH],
                    axis=mybir.AxisListType.X,
                    op=mybir.AluOpType.add,
                )
            nc.sync.dma_start(out_d[:], outsb[:])

    nc.compile()
    return nc


def _get_nc():
    if "nc" not in _CACHE:
        _CACHE["nc"] = _build_nc()
    return _CACHE["nc"]


def _run_device(y_true, norm_logits, trace=False, trace_cores=None):
    from concourse import bass_utils

    nc = _get_nc()
    x = np.ascontiguousarray(np.asarray(norm_logits, dtype=np.float32))
    y = np.ascontiguousarray(np.asarray(y_true, dtype=np.float32))
    in_maps = [
        {
            "x": np.ascontiguousarray(x[:, k * CS : (k + 1) * CS]),
            "y": np.ascontiguousarray(y[:, k * CS : (k + 1) * CS]),
        }
        for k in range(NCORES)
    ]
    kwargs = {}
    if trace:
        kwargs["trace"] = True
        kwargs["trace_cores"] = (
            list(range(NCORES)) if trace_cores is None else trace_cores
        )
    return bass_utils.run_bass_kernel_spmd(
        nc, in_maps, core_ids=list(range(NCORES)), **kwargs
    )


def _combine(core_outs):
    """Unshard: sum per-core [128, 12] partials and apply the scalar tail."""
    arr = np.stack([np.asarray(o, dtype=np.float64) for o in core_outs])  # [8,128,12]
    # column p of row group r holds global row r*128 + p -> transpose to [RG, P]
    hvh = arr[:, :, 0:RG].sum(axis=0).T.reshape(-1)  # [512]
    se = arr[:, :, RG : 2 * RG].sum(axis=0).T.reshape(-1)  # [512]
    col0 = arr[0, :, 2 * RG : 3 * RG].T.reshape(-1)  # [512] (global col 0 = core 0)

    hit = hvh > KOFF / 2  # exactly one hit: hvh = v + 16 in [15.01, 16.99]
    v = np.where(hit, hvh - KOFF, col0)
    t = np.cos(np.arccos(np.clip(v, -1.0, 1.0)) + M2)
    tv = np.where(v > THRESHOLD, t, -2.0 - t)
    S = se + hit * (np.exp(SCALE * tv - SCALE) - np.exp(SCALE * v - SCALE))
    loss_rows = hit * (SCALE + np.log(S) - SCALE * tv)
    return np.asarray(loss_rows.mean(), dtype=np.float32)


def kernel(y_true, norm_logits):
    res = _run_device(y_true, norm_logits)
    return _combine([r["out"] for r in res.results])


# revision 22
# speedup vs baseline: 1.5231x; 1.5231x over previous
"""ArcFace loss on 8 TRN2 NeuronCores — class-dimension (C) sharded.

Math (reference has M1=1, M2=0.5, M3=0, scale=64, label_smoothing=0):
  per row i with one-hot y_true:  v_i = x[i, label_i] = sum_j y[i,j]*x[i,j]
  t_i = cos(acos(v_i) + 0.5),  t_i -> -2 - t_i when v_i <= cos(pi - 0.5)
  loss_i = logsumexp_j(64 * modified_x[i,j]) - 64*t_i
  loss = mean_i loss_i          (0 when a row of y_true is all zero)

All logits lie in (-0.99, 0.99), so 64*x - 64 <= 0 and a FIXED shift of 64
replaces the row-max in logsumexp (no max pass, no second streaming pass):
  logsumexp_i = 64 + log(S_i),
  S_i = sum_j exp(64*x[i,j] - 64) + exp(64*t_i - 64) - exp(64*v_i - 64)

Each core streams its [512, 12500] shard of x (f32) and y (staged as uint8 —
lossless for an exact {0,1} one-hot, and 4x fewer bytes) once and emits
per-row partials:
  hvh_i = sum_j (x[i,j] + 16) * y[i,j]   (= v_i + 16 if the label is local,
                                          exactly 0 otherwise — encodes both
                                          the hit flag and the hit value)
  se_i  = sum_j exp(64*x[i,j] - 64)
plus column 0 of the local shard (needed to mimic argmax(all-zeros)=0 when a
y_true row is entirely zero — the reference then returns a 0 contribution,
so col0 is only used to keep the formulas well-defined).

The host "unshard" step sums the [512]-sized partials over the 8 cores and
applies the closed-form tail (acos/cos/log on 512 scalars).
"""

import os

import numpy as np

B = 512
C = 100000
NCORES = 8
CS = C // NCORES  # 12500 classes per core
P = 128
RG = B // P  # 4 row groups of 128 partitions
FCH = int(os.environ.get("AK_FCH", "6250"))  # free-dim chunk
NCH = CS // FCH  # chunks per row group
XBUFS = int(os.environ.get("AK_XBUFS", "2"))
YBUFS = int(os.environ.get("AK_YBUFS", "2"))
EBUFS = int(os.environ.get("AK_EBUFS", "2"))
YENG = os.environ.get("AK_YENG", "sync")  # engine issuing y-shard loads
EOUT = os.environ.get("AK_EOUT", "scratch")  # exp 'out' target: scratch|dummy|inplace
# y_true is an exact {0.0, 1.0} one-hot, so staging it as uint8 is lossless
# (the DVE converts u8 -> fp32 0/1 in-datapath; results are bit-identical to
# f32-staged y in every measured run) and cuts the streamed bytes from
# 51.2 MB to 32 MB per core.  x stays f32 for full precision; "bf16" staging
# of x is supported (another 1.35x, measured rel err ~7e-5) but off by default.
YDTYPE = os.environ.get("AK_YDTYPE", "u8")  # y staging dtype: f32|u8|u8cast
XDTYPE = os.environ.get("AK_XDTYPE", "f32")  # x staging dtype: f32|bf16
YFCH = int(os.environ.get("AK_YFCH", str(FCH)))  # y free-dim chunk (multiple of FCH)
assert YFCH % FCH == 0 and CS % YFCH == 0
TAILSPLIT = os.environ.get("AK_TAILSPLIT", "1") == "1"  # halve the final chunk twice

KOFF = 16.0  # hit-encoding offset: hvh = v + 16 iff label is in-shard
SCALE = 64.0
M2 = 0.5
THRESHOLD = float(np.cos(np.pi - M2))

_CACHE = {}


def _build_nc():
    import concourse.tile as tile
    from concourse import bacc, mybir

    nc = bacc.Bacc(
        "TRN2",
        target_bir_lowering=False,
        debug=False,
        enable_asserts=False,
        num_devices=NCORES,
    )
    f32 = mybir.dt.float32
    y_dt = f32 if YDTYPE == "f32" else mybir.dt.uint8
    x_dt = f32 if XDTYPE == "f32" else mybir.dt.bfloat16
    x_d = nc.dram_tensor("x", [B, CS], x_dt, kind="ExternalInput").ap()
    y_d = nc.dram_tensor("y", [B, CS], y_dt, kind="ExternalInput").ap()
    # out columns: [0:RG] hvh per row group, [RG:2RG] se, [2RG:3RG] shard col0
    out_d = nc.dram_tensor("out", [P, 3 * RG], f32, kind="ExternalOutput").ap()

    with tile.TileContext(nc) as tc:
        with (
            tc.tile_pool(name="xin", bufs=XBUFS) as xpool,
            tc.tile_pool(name="yin", bufs=YBUFS) as ypool,
            tc.tile_pool(name="escratch", bufs=EBUFS) as epool,
            tc.tile_pool(name="stats", bufs=1) as stats,
        ):
            y_dma = getattr(nc, YENG)
            hvh_parts = stats.tile([P, RG * NCH + 1], f32)
            se_parts = stats.tile([P, RG * NCH + 1], f32)
            outsb = stats.tile([P, 3 * RG], f32)
            dummy = stats.tile([P, 1], f32)
            dummy2 = stats.tile([P, 1], f32)
            neg_scale = stats.tile([P, 1], f32)
            nc.vector.memset(neg_scale[:], -SCALE)

            yt_dt = f32 if YDTYPE in ("f32", "u8cast") else mybir.dt.uint8
            y_loader = nc.gpsimd if YDTYPE == "u8cast" else y_dma
            i = 0  # global partial-column index
            for r in range(RG):
                widths = [FCH] * NCH
                if TAILSPLIT and r == RG - 1:
                    # shrink the final chunks so less compute trails the last DMA
                    widths = [FCH] * (NCH - 1) + [FCH // 2, FCH // 2]
                i0, off, yt, ybase = i, 0, None, -1
                for w in widths:
                    xt = xpool.tile([P, FCH], x_dt, tag="xt")
                    nc.sync.dma_start(
                        xt[:, :w], x_d[r * P : (r + 1) * P, off : off + w]
                    )
                    if off // YFCH != ybase:
                        ybase = off // YFCH
                        yt = ypool.tile([P, YFCH], yt_dt, tag="yt")
                        # u8cast: SWDGE casts u8->f32 during the DMA itself
                        y_loader.dma_start(
                            yt[:],
                            y_d[r * P : (r + 1) * P, ybase * YFCH : (ybase + 1) * YFCH],
                        )
                    yc = off - ybase * YFCH
                    # DVE: hvh partial = sum((x + 16) * y) along the chunk
                    nc.vector.scalar_tensor_tensor(
                        out=dummy.broadcast_to([P, w]),
                        in0=xt[:, :w],
                        scalar=KOFF,
                        in1=yt[:, yc : yc + w],
                        op0=mybir.AluOpType.add,
                        op1=mybir.AluOpType.mult,
                        accum_out=hvh_parts[:, i : i + 1],
                    )
                    # ACT: se partial = sum(exp(64*x - 64)) along the chunk
                    if EOUT == "dummy":
                        et_ap = dummy2.broadcast_to([P, w])
                    elif EOUT == "inplace":
                        et_ap = xt[:, :w]
                    else:
                        et = epool.tile([P, FCH], f32, tag="et")
                        et_ap = et[:, :w]
                    nc.scalar.activation(
                        out=et_ap,
                        in_=xt[:, :w],
                        func=mybir.ActivationFunctionType.Exp,
                        bias=neg_scale[:],
                        scale=SCALE,
                        accum_out=se_parts[:, i : i + 1],
                    )
                    if off == 0:
                        nc.vector.tensor_copy(
                            outsb[:, 2 * RG + r : 2 * RG + r + 1], xt[:, 0:1]
                        )
                    off += w
                    i += 1
                # per-group combine right after the group's chunks
                nc.vector.tensor_reduce(
                    out=outsb[:, r : r + 1],
                    in_=hvh_parts[:, i0:i],
                    axis=mybir.AxisListType.X,
                    op=mybir.AluOpType.add,
                )
                nc.vector.tensor_reduce(
                    out=outsb[:, RG + r : RG + r + 1],
                    in_=se_parts[:, i0:i],
                    axis=mybir.AxisListType.X,
                    op=mybir.AluOpType.add,
                )
            nc.sync.dma_start(out_d[:], outsb[:])

    nc.compile()
    return nc


def _get_nc():
    if "nc" not in _CACHE:
        _CACHE["nc"] = _build_nc()
    return _CACHE["nc"]


def _run_device(y_true, norm_logits, trace=False, trace_cores=None):
    from concourse import bass_utils

    nc = _get_nc()
    x = np.ascontiguousarray(np.asarray(norm_logits, dtype=np.float32))
    y = np.ascontiguousarray(np.asarray(y_true, dtype=np.float32))
    y_np = np.float32 if YDTYPE == "f32" else np.uint8
    if XDTYPE == "f32":
        x_np = np.float32
    else:
        import ml_dtypes

        x_np = ml_dtypes.bfloat16
    in_maps = [
        {
            "x": np.ascontiguousarray(x[:, k * CS : (k + 1) * CS]).astype(x_np),
            "y": np.ascontiguousarray(y[:, k * CS : (k + 1) * CS]).astype(y_np),
        }
        for k in range(NCORES)
    ]
    kwargs = {}
    if trace:
        kwargs["trace"] = True
        kwargs["trace_cores"] = (
            list(range(NCORES)) if trace_cores is None else trace_cores
        )
    return bass_utils.run_bass_kernel_spmd(
        nc, in_maps, core_ids=list(range(NCORES)), **kwargs
    )


def _combine(core_outs):
    """Unshard: sum per-core [128, 12] partials and apply the scalar tail."""
    arr = np.stack([np.asarray(o, dtype=np.float64) for o in core_outs])  # [8,128,12]
    # column p of row group r holds global row r*128 + p -> transpose to [RG, P]
    hvh = arr[:, :, 0:RG].sum(axis=0).T.reshape(-1)  # [512]
    se = arr[:, :, RG : 2 * RG].sum(axis=0).T.reshape(-1)  # [512]
    col0 = arr[0, :, 2 * RG : 3 * RG].T.reshape(-1)  # [512] (global col 0 = core 0)

    hit = hvh > KOFF / 2  # exactly one hit: hvh = v + 16 in [15.01, 16.99]
    v = np.where(hit, hvh - KOFF, col0)
    t = np.cos(np.arccos(np.clip(v, -1.0, 1.0)) + M2)
    tv = np.where(v > THRESHOLD, t, -2.0 - t)
    S = se + hit * (np.exp(SCALE * tv - SCALE) - np.exp(SCALE * v - SCALE))
    loss_rows = hit * (SCALE + np.log(S) - SCALE * tv)
    return np.asarray(loss_rows.mean(), dtype=np.float32)


def kernel(y_true, norm_logits):
    res = _run_device(y_true, norm_logits)
    return _combine([r["out"] for r in res.results])

